# revision 2
# baseline (speedup 1.0000x reference)
"""LocalGCN message-passing kernel, data-parallel over 8 NeuronCores.

Sharding (per spec hint): pure data parallel — the batch dim (B=32768) is
split into 8 shards of 4096 rows, one per core; all parameters are
replicated; each row's 8-neighbor attention is independent so there is no
cross-core communication. Inputs arrive FULL, are sharded host-side,
executed SPMD on the 8 cores, and the outputs are gathered back to the
full [32768, 128] array.
"""
import os

# Faster neuronxcc compile; must be set before the jax backend initializes.
_flags = os.environ.get("NEURON_CC_FLAGS", "")
if "--optlevel" not in _flags and "-O" not in _flags:
    os.environ["NEURON_CC_FLAGS"] = (_flags + " --optlevel=1").strip()

import numpy as np
import jax
import jax.numpy as jnp
from functools import partial

# Hardcoded problem shape (nn_LocalGCN_70489003262550)
D_IN, HID, HEADS, MAXN, OUT, B = 16, 128, 4, 8, 128, 32768
HD = HID // HEADS
EPS = 1e-5
N_CORES = 8
B_SH = B // N_CORES  # 4096 rows per core

PARAM_NAMES = (
    "enc_w1", "enc_b1", "enc_g1", "enc_be1", "enc_w2", "enc_b2", "enc_g2",
    "enc_be2", "in_proj_w", "in_proj_b", "out_w", "out_b", "an_g", "an_b",
    "p1_w", "p1_b", "p1_g", "p1_be", "p2_w", "p2_b", "p2_g", "p2_be",
)


def _ln(x, g, b):
    m = jnp.mean(x, -1, keepdims=True)
    v = jnp.mean((x - m) ** 2, -1, keepdims=True)
    return (x - m) * jax.lax.rsqrt(v + EPS) * g + b


def _encode(x, p):
    h = _ln(x @ p["enc_w1"].T + p["enc_b1"], p["enc_g1"], p["enc_be1"])
    h = jax.nn.relu(h)
    return _ln(h @ p["enc_w2"].T + p["enc_b2"], p["enc_g2"], p["enc_be2"])


def _shard_forward(current_node, neighbor_nodes, neighbor_mask, *params):
    p = dict(zip(PARAM_NAMES, params))
    bsz = current_node.shape[0]
    cur_emb = _encode(current_node, p)                                # [b, HID]
    nb_emb = _encode(neighbor_nodes, p)                               # [b, N, HID]

    valid = neighbor_mask > 0                                         # [b, N]
    has_nb = jnp.any(valid, axis=1)                                   # [b]

    wq, wk, wv = (p["in_proj_w"][:HID], p["in_proj_w"][HID:2 * HID],
                  p["in_proj_w"][2 * HID:])
    bq, bk, bv = (p["in_proj_b"][:HID], p["in_proj_b"][HID:2 * HID],
                  p["in_proj_b"][2 * HID:])
    q = (cur_emb @ wq.T + bq).reshape(bsz, HEADS, HD)
    k = (nb_emb @ wk.T + bk).reshape(bsz, MAXN, HEADS, HD)
    v = (nb_emb @ wv.T + bv).reshape(bsz, MAXN, HEADS, HD)

    scores = jnp.einsum("bhd,bnhd->bhn", q, k) / np.sqrt(HD)          # [b, H, N]
    safe = jnp.where(has_nb[:, None], valid, True)                    # [b, N]
    scores = jnp.where(safe[:, None, :], scores, -1e9)
    attn = jax.nn.softmax(scores, axis=-1)
    ctx = jnp.einsum("bhn,bnhd->bhd", attn, v).reshape(bsz, HID)
    ctx = ctx @ p["out_w"].T + p["out_b"]
    agg = jnp.where(has_nb[:, None], _ln(ctx, p["an_g"], p["an_b"]), cur_emb)

    combined = jnp.concatenate([cur_emb, agg], axis=-1)               # [b, 2*HID]
    h = _ln(combined @ p["p1_w"].T + p["p1_b"], p["p1_g"], p["p1_be"])
    h = jax.nn.relu(h)
    return _ln(h @ p["p2_w"].T + p["p2_b"], p["p2_g"], p["p2_be"])    # [b, OUT]


_pmapped = None


def _get_pmapped():
    global _pmapped
    if _pmapped is None:
        # batch tensors sharded on axis 0; all params replicated (broadcast)
        _pmapped = jax.pmap(
            _shard_forward,
            in_axes=(0, 0, 0) + (None,) * len(PARAM_NAMES),
            devices=jax.devices()[:N_CORES],
        )
    return _pmapped


def kernel(**inputs) -> np.ndarray:
    cur = np.asarray(inputs["current_node"], np.float32).reshape(
        N_CORES, B_SH, D_IN)
    nb = np.asarray(inputs["neighbor_nodes"], np.float32).reshape(
        N_CORES, B_SH, MAXN, D_IN)
    mask = np.asarray(inputs["neighbor_mask"], np.int32).reshape(
        N_CORES, B_SH, MAXN)
    params = tuple(np.asarray(inputs[n], np.float32) for n in PARAM_NAMES)

    fn = _get_pmapped()
    out = fn(cur, nb, mask, *params)             # [8, 4096, OUT]
    return np.asarray(out).reshape(B, OUT).astype(np.float32)


if __name__ == "__main__":
    rng = np.random.default_rng(0)
    demo = {
        "current_node": rng.standard_normal((B, D_IN), np.float32),
        "neighbor_nodes": rng.standard_normal((B, MAXN, D_IN), np.float32),
        "neighbor_mask": rng.integers(0, 2, (B, MAXN)).astype(np.int32),
    }
    for n in PARAM_NAMES:
        pass  # weights needed; run via test.py instead


# revision 3
# speedup vs baseline: 1.2870x; 1.2870x over previous
"""LocalGCN message-passing kernel, data-parallel over 8 NeuronCores.

Sharding (per spec hint): pure data parallel — the batch dim (B=32768) is
split into 8 shards of 4096 rows, one per core; all parameters are
replicated; each row's 8-neighbor attention is independent so there is no
cross-core communication. Inputs arrive FULL, are sharded host-side,
executed SPMD on the 8 cores, and the outputs are gathered back to the
full [32768, 128] array.
"""
import os

# Faster neuronxcc compile; must be set before the jax backend initializes.
_flags = os.environ.get("NEURON_CC_FLAGS", "")
if "--optlevel" not in _flags and "-O" not in _flags:
    os.environ["NEURON_CC_FLAGS"] = (_flags + " --optlevel=1").strip()

import numpy as np
import jax
import jax.numpy as jnp
from functools import partial

# Hardcoded problem shape (nn_LocalGCN_70489003262550)
D_IN, HID, HEADS, MAXN, OUT, B = 16, 128, 4, 8, 128, 32768
HD = HID // HEADS
EPS = 1e-5
N_CORES = 8
B_SH = B // N_CORES  # 4096 rows per core

PARAM_NAMES = (
    "enc_w1", "enc_b1", "enc_g1", "enc_be1", "enc_w2", "enc_b2", "enc_g2",
    "enc_be2", "in_proj_w", "in_proj_b", "out_w", "out_b", "an_g", "an_b",
    "p1_w", "p1_b", "p1_g", "p1_be", "p2_w", "p2_b", "p2_g", "p2_be",
)


def _ln(x, g, b):
    m = jnp.mean(x, -1, keepdims=True)
    v = jnp.mean((x - m) ** 2, -1, keepdims=True)
    return (x - m) * jax.lax.rsqrt(v + EPS) * g + b


def _encode(x, p):
    h = _ln(x @ p["enc_w1"].T + p["enc_b1"], p["enc_g1"], p["enc_be1"])
    h = jax.nn.relu(h)
    return _ln(h @ p["enc_w2"].T + p["enc_b2"], p["enc_g2"], p["enc_be2"])


def _shard_forward(current_node, neighbor_nodes, neighbor_mask, *params):
    p = dict(zip(PARAM_NAMES, params))
    bsz = current_node.shape[0]
    cur_emb = _encode(current_node, p)                                # [b, HID]
    nb_emb = _encode(neighbor_nodes, p)                               # [b, N, HID]

    valid = neighbor_mask > 0                                         # [b, N]
    has_nb = jnp.any(valid, axis=1)                                   # [b]

    wq, wk, wv = (p["in_proj_w"][:HID], p["in_proj_w"][HID:2 * HID],
                  p["in_proj_w"][2 * HID:])
    bq, bk, bv = (p["in_proj_b"][:HID], p["in_proj_b"][HID:2 * HID],
                  p["in_proj_b"][2 * HID:])
    q = (cur_emb @ wq.T + bq).reshape(bsz, HEADS, HD)
    k = (nb_emb @ wk.T + bk).reshape(bsz, MAXN, HEADS, HD)
    v = (nb_emb @ wv.T + bv).reshape(bsz, MAXN, HEADS, HD)

    scores = jnp.einsum("bhd,bnhd->bhn", q, k) / np.sqrt(HD)          # [b, H, N]
    safe = jnp.where(has_nb[:, None], valid, True)                    # [b, N]
    scores = jnp.where(safe[:, None, :], scores, -1e9)
    attn = jax.nn.softmax(scores, axis=-1)
    ctx = jnp.einsum("bhn,bnhd->bhd", attn, v).reshape(bsz, HID)
    ctx = ctx @ p["out_w"].T + p["out_b"]
    agg = jnp.where(has_nb[:, None], _ln(ctx, p["an_g"], p["an_b"]), cur_emb)

    combined = jnp.concatenate([cur_emb, agg], axis=-1)               # [b, 2*HID]
    h = _ln(combined @ p["p1_w"].T + p["p1_b"], p["p1_g"], p["p1_be"])
    h = jax.nn.relu(h)
    return _ln(h @ p["p2_w"].T + p["p2_b"], p["p2_g"], p["p2_be"])    # [b, OUT]


_cache = {"fp": None, "fn": None}


def _get_pmapped(params):
    # Bake the (small, replicated) parameters into the executable as
    # constants: avoids 22 params x 8 devices of per-call H2D round trips
    # over the tunneled PJRT link. Re-traces only if param values change.
    fp = hash(tuple(p.tobytes() for p in params))
    if _cache["fp"] != fp:
        const = {n: jnp.asarray(p) for n, p in zip(PARAM_NAMES, params)}

        def fwd(cur, nb, mask):
            return _shard_forward(cur, nb, mask,
                                  *(const[n] for n in PARAM_NAMES))

        _cache["fn"] = jax.pmap(fwd, devices=jax.devices()[:N_CORES])
        _cache["fp"] = fp
    return _cache["fn"]


def kernel(**inputs) -> np.ndarray:
    cur = np.asarray(inputs["current_node"], np.float32).reshape(
        N_CORES, B_SH, D_IN)
    nb = np.asarray(inputs["neighbor_nodes"], np.float32).reshape(
        N_CORES, B_SH, MAXN, D_IN)
    mask = np.asarray(inputs["neighbor_mask"], np.int32).reshape(
        N_CORES, B_SH, MAXN)
    params = tuple(np.asarray(inputs[n], np.float32) for n in PARAM_NAMES)

    fn = _get_pmapped(params)
    out = fn(cur, nb, mask)                      # [8, 4096, OUT]
    return np.asarray(out).reshape(B, OUT)


if __name__ == "__main__":
    rng = np.random.default_rng(0)
    demo = {
        "current_node": rng.standard_normal((B, D_IN), np.float32),
        "neighbor_nodes": rng.standard_normal((B, MAXN, D_IN), np.float32),
        "neighbor_mask": rng.integers(0, 2, (B, MAXN)).astype(np.int32),
    }
    for n in PARAM_NAMES:
        pass  # weights needed; run via test.py instead


# revision 4
# speedup vs baseline: 1.3077x; 1.0161x over previous
"""LocalGCN message-passing kernel, data-parallel over 8 NeuronCores.

Sharding (per spec hint): pure data parallel — the batch dim (B=32768) is
split into 8 shards of 4096 rows, one per core; all parameters are
replicated; each row's 8-neighbor attention is independent so there is no
cross-core communication. Inputs arrive FULL, are sharded host-side,
executed SPMD on the 8 cores, and the outputs are gathered back to the
full [32768, 128] array.
"""
import os

# Faster neuronxcc compile; must be set before the jax backend initializes.
_flags = os.environ.get("NEURON_CC_FLAGS", "")
if "--optlevel" not in _flags and "-O" not in _flags:
    os.environ["NEURON_CC_FLAGS"] = (_flags + " --optlevel=2").strip()

import numpy as np
import jax
import jax.numpy as jnp
from functools import partial

# Hardcoded problem shape (nn_LocalGCN_70489003262550)
D_IN, HID, HEADS, MAXN, OUT, B = 16, 128, 4, 8, 128, 32768
HD = HID // HEADS
EPS = 1e-5
N_CORES = 8
B_SH = B // N_CORES  # 4096 rows per core

PARAM_NAMES = (
    "enc_w1", "enc_b1", "enc_g1", "enc_be1", "enc_w2", "enc_b2", "enc_g2",
    "enc_be2", "in_proj_w", "in_proj_b", "out_w", "out_b", "an_g", "an_b",
    "p1_w", "p1_b", "p1_g", "p1_be", "p2_w", "p2_b", "p2_g", "p2_be",
)


def _ln(x, g, b):
    m = jnp.mean(x, -1, keepdims=True)
    v = jnp.mean((x - m) ** 2, -1, keepdims=True)
    return (x - m) * jax.lax.rsqrt(v + EPS) * g + b


def _encode(x, p):
    h = _ln(x @ p["enc_w1"].T + p["enc_b1"], p["enc_g1"], p["enc_be1"])
    h = jax.nn.relu(h)
    return _ln(h @ p["enc_w2"].T + p["enc_b2"], p["enc_g2"], p["enc_be2"])


def _shard_forward(current_node, neighbor_nodes, neighbor_mask, *params):
    p = dict(zip(PARAM_NAMES, params))
    bsz = current_node.shape[0]
    cur_emb = _encode(current_node, p)                                # [b, HID]
    nb_emb = _encode(neighbor_nodes, p)                               # [b, N, HID]

    valid = neighbor_mask > 0                                         # [b, N]
    has_nb = jnp.any(valid, axis=1)                                   # [b]

    wq, wk, wv = (p["in_proj_w"][:HID], p["in_proj_w"][HID:2 * HID],
                  p["in_proj_w"][2 * HID:])
    bq, bk, bv = (p["in_proj_b"][:HID], p["in_proj_b"][HID:2 * HID],
                  p["in_proj_b"][2 * HID:])
    q = (cur_emb @ wq.T + bq).reshape(bsz, HEADS, HD)
    k = (nb_emb @ wk.T + bk).reshape(bsz, MAXN, HEADS, HD)
    v = (nb_emb @ wv.T + bv).reshape(bsz, MAXN, HEADS, HD)

    scores = jnp.einsum("bhd,bnhd->bhn", q, k) / np.sqrt(HD)          # [b, H, N]
    safe = jnp.where(has_nb[:, None], valid, True)                    # [b, N]
    scores = jnp.where(safe[:, None, :], scores, -1e9)
    attn = jax.nn.softmax(scores, axis=-1)
    ctx = jnp.einsum("bhn,bnhd->bhd", attn, v).reshape(bsz, HID)
    ctx = ctx @ p["out_w"].T + p["out_b"]
    agg = jnp.where(has_nb[:, None], _ln(ctx, p["an_g"], p["an_b"]), cur_emb)

    combined = jnp.concatenate([cur_emb, agg], axis=-1)               # [b, 2*HID]
    h = _ln(combined @ p["p1_w"].T + p["p1_b"], p["p1_g"], p["p1_be"])
    h = jax.nn.relu(h)
    return _ln(h @ p["p2_w"].T + p["p2_b"], p["p2_g"], p["p2_be"])    # [b, OUT]


_cache = {"fp": None, "fn": None}


def _get_pmapped(params):
    # Bake the (small, replicated) parameters into the executable as
    # constants: avoids 22 params x 8 devices of per-call H2D round trips
    # over the tunneled PJRT link. Re-traces only if param values change.
    fp = hash(tuple(p.tobytes() for p in params))
    if _cache["fp"] != fp:
        const = {n: jnp.asarray(p) for n, p in zip(PARAM_NAMES, params)}

        def fwd(cur, nb, mask):
            return _shard_forward(cur, nb, mask,
                                  *(const[n] for n in PARAM_NAMES))

        _cache["fn"] = jax.pmap(fwd, devices=jax.devices()[:N_CORES])
        _cache["fp"] = fp
    return _cache["fn"]


def kernel(**inputs) -> np.ndarray:
    cur = np.asarray(inputs["current_node"], np.float32).reshape(
        N_CORES, B_SH, D_IN)
    nb = np.asarray(inputs["neighbor_nodes"], np.float32).reshape(
        N_CORES, B_SH, MAXN, D_IN)
    mask = np.asarray(inputs["neighbor_mask"], np.int32).reshape(
        N_CORES, B_SH, MAXN)
    params = tuple(np.asarray(inputs[n], np.float32) for n in PARAM_NAMES)

    fn = _get_pmapped(params)
    out = fn(cur, nb, mask)                      # [8, 4096, OUT]
    return np.asarray(out).reshape(B, OUT)


if __name__ == "__main__":
    rng = np.random.default_rng(0)
    demo = {
        "current_node": rng.standard_normal((B, D_IN), np.float32),
        "neighbor_nodes": rng.standard_normal((B, MAXN, D_IN), np.float32),
        "neighbor_mask": rng.integers(0, 2, (B, MAXN)).astype(np.int32),
    }
    for n in PARAM_NAMES:
        pass  # weights needed; run via test.py instead
